# revision 17
# baseline (speedup 1.0000x reference)
"""Single-token GQA decode attention (32 q heads / 8 kv heads, 8192-pos KV
cache, dim 4096) tensor-parallel over 8 NeuronCores.

Sharding (per core c): q heads [4c, 4c+4), kv head c; x replicated; each
core computes a full-width [128, 32] partial of the output projection
(out[n] lives at [n % 128, n // 128]); partials are summed and transposed
host-side.

Layouts are chosen so every matmul is stationary-weight with a tiny moving
side (out free size 1-4, ~free on the PE):
  - wq  fp16 [128, 16384]: block (c, qb) at cols c*512+qb*128 is the
    [x-dim 128, q-dim 128] stationary tile; pqT accumulates q in column
    (transposed) layout [128 d, 4 h] directly.
  - wkv fp8e3 x64 [128, 8192]: same per-c blocks for the new k|v columns.
  - wo  fp8e3 x64 [128, 16384]: block (m, g) at cols m*512+g*128 maps
    attn column g to outputs [m*128, (m+1)*128).
  - K   fp16 [128, 8192] transposed cache, V fp16 [128, 64, 128]
    partition-swizzled cache (fp16: K/V quantization noise is amplified
    through the softmax exponent / feeds the output directly).
RoPE is applied in column space: q_rot = cos_col * q + sin_col' * (P q)
with P a constant pair-swap matrix (one matmul + 3 DVE ops); the k-new
rotation folds the 1/64 fp8 weight scale into its cos/sin columns.

The new token is decoupled from the cache stream: the stale cache slot at
position 8191 (rolled to partition 0 of the last chunk host-side) flows
through scores/exp and its e is zeroed; the real new-token contribution
enters via rank-1 matmuls (e_new, pav += xv^T e_new, pz += 64*e_new).

DMA: the cost model charges each engine ring independently, so the load
stream is striped across the three DMA-capable rings (SP, Activation,
Pool/gpsimd), each ~1/3 of the ~88KB of per-partition line bytes.  All
small constants ship as one [128, 170] fp16 tensor.  ACT carries less DMA
because it owns LoadActFuncSet and the exp ops, which must slot between
its stripes before AV.
"""

import numpy as np
import ml_dtypes

import concourse.tile as tile
from concourse import bacc, mybir
from concourse.bass_utils import run_bass_kernel_spmd
from concourse.tile import add_dep_helper

N_CORES = 8
DIM = 4096
HEAD_DIM = 128
N_HEADS = 32
N_KV_HEADS = 8
REPEATS = N_HEADS // N_KV_HEADS  # 4 q heads per core
KV_LEN = 8192                    # start_pos + 1
NQ = REPEATS * HEAD_DIM          # 512 local q dims
NKV = 2 * HEAD_DIM               # 256 local k|v dims
KCH = DIM // 128                 # 32 contraction chunks
TCH = KV_LEN // 128              # 64 kv-position chunks
NB = DIM // 128                  # 32 output n-blocks
WS = 64.0                        # fp8 weight scale (power of 2, exact)
SCALE = 1.0 / np.sqrt(np.float32(HEAD_DIM))

F32 = mybir.dt.float32
F16 = mybir.dt.float16
F8 = mybir.dt.float8e3
E3M4 = ml_dtypes.float8_e3m4

# const tensor column layout (fp16 [128, CONST_COLS])
C_PSWP = 0            # [128, 128] pair-swap permutation
C_CQ = 128            # [128, 4] cos column for q
C_SQ = 132            # [128, 4] signed sin column for q
C_CK = 136            # [128, 1] cos column / 64 for new k
C_SK = 137            # [128, 1] signed sin column / 64 for new k
C_X = 138             # [128, 32] x columns
C_ID = 170            # [128, 128] identity (moving side of col->row transpose)
CONST_COLS = 298

# DMA striping: per tensor, list of (ring, lo, hi) column ranges.
# ring 0 = SP, 1 = ACT, 2 = Pool.  Budgets (line KB): wq 32 f16, wkv 8 f8,
# K 16 f16, V 16 f16, wo 16 f8; ACT carries less (LAFS + exps).
STRIPES = {
    # line-KB budgets: SP ~30.5, ACT ~26 (owns LAFS+exps), Pool ~31.5
    # wq cols (of 16384, f16): SP 12KB, ACT 9KB, Pool 11KB
    "wq": [(0, 0, 6144), (1, 6144, 10752), (2, 10752, 16384)],
    # wkv (8192 cols f8): Pool only (early, feeds new-token path)
    "wkv": [(2, 0, 8192)],
    # K cols (of 8192, f16): SP 7KB, ACT 6KB, Pool 3KB
    "k": [(0, 0, 3584), (1, 3584, 6656), (2, 6656, 8192)],
    # V pos-cols (of 8192, f16): SP 6KB, ACT 6KB, Pool 4KB
    "v": [(0, 0, 3072), (1, 3072, 6144), (2, 6144, 8192)],
    # wo cols (of 16384, f8): SP 5.5KB, ACT 5KB (deferred past exps), Pool 5.5KB
    "wo": [(0, 0, 5632), (1, 5632, 10752), (2, 10752, 16384)],
}

_CACHED = {}


def _build(reps=1):
    nc = bacc.Bacc(None, target_bir_lowering=False)

    consts = nc.dram_tensor("consts", [128, CONST_COLS], F16, kind="ExternalInput")
    wq_m = nc.dram_tensor("wq_m", [128, KCH * NQ], F16, kind="ExternalInput")
    wkv8 = nc.dram_tensor("wkv8", [128, KCH * NKV], F8, kind="ExternalInput")
    wo8 = nc.dram_tensor("wo8", [128, 4 * DIM], F8, kind="ExternalInput")
    k_t = nc.dram_tensor("k_t", [128, KV_LEN], F16, kind="ExternalInput")
    v_s = nc.dram_tensor("v_s", [128, TCH * 128], F16, kind="ExternalInput")
    out_p = nc.dram_tensor("out_p", [128, NB], F32, kind="ExternalOutput")

    with tile.TileContext(nc) as tc:
        with (
            tc.tile_pool(name="small", bufs=1) as small,
            tc.tile_pool(name="big", bufs=1) as big,
        ):
          for _rep in range(reps):
            rings = [nc.sync, nc.scalar, nc.gpsimd]
            tails = [None, None, None]

            def chain(ring, inst):
                if tails[ring] is not None:
                    add_dep_helper(inst.ins, tails[ring].ins, sync=False,
                                   reason="ring order")
                tails[ring] = inst

            def stripe_dma(key, sbuf, dram):
                for ring, lo, hi in STRIPES[key]:
                    chain(ring, rings[ring].dma_start(
                        out=sbuf[:, lo:hi], in_=dram[:, lo:hi]))

            c_sb = small.tile([128, CONST_COLS], F16)
            wq_sb = big.tile([128, KCH * NQ], F16)
            wkv_sb = big.tile([128, KCH * NKV], F8)
            kt_sb = big.tile([128, KV_LEN], F16)
            v_sb = big.tile([128, TCH * 128], F16)
            wo_sb = big.tile([128, 4 * DIM], F8)

            # ACT: dummy activation first so LoadActFuncSet lands at t~0
            id1 = small.tile([1, 1], F16)
            nc.vector.memset(id1[:], 1.0)
            scr1 = small.tile([1, 1], F16)
            nc.scalar.activation(scr1[:], id1[:],
                                 mybir.ActivationFunctionType.Copy)

            # the three parallel load rings
            chain(0, nc.sync.dma_start(out=c_sb[:], in_=consts[:]))
            stripe_dma("wkv", wkv_sb, wkv8)
            stripe_dma("wq", wq_sb, wq_m)
            stripe_dma("k", kt_sb, k_t)
            stripe_dma("v", v_sb, v_s)
            # ACT's wo stripe is emitted after the exp ops (ACT executes in
            # program order; exps must not queue behind it)
            for ring, lo, hi in STRIPES["wo"]:
                if ring != 1:
                    chain(ring, rings[ring].dma_start(
                        out=wo_sb[:, lo:hi], in_=wo8[:, lo:hi]))

            x_sb = c_sb[:, C_X:C_X + KCH]

            ones_sb = small.tile([128, 1], F32)
            nc.vector.memset(ones_sb[:], WS)     # folds 1/64 into 1/pz
            ones1 = small.tile([1, 1], F16)
            nc.vector.memset(ones1[:], WS)
            ones_row = small.tile([1, 128], F32)
            nc.vector.memset(ones_row[:], 1.0)

            qTpre = small.tile([128, REPEATS], F16)
            swq = small.tile([128, REPEATS], F16)
            qT = small.tile([128, REPEATS], F16)
            kcol = small.tile([128, 1], F16)
            swk = small.tile([128, 1], F16)
            knT = small.tile([128, 1], F16)
            vcol = small.tile([128, 1], F16)
            xv_sb = small.tile([1, HEAD_DIM], F16)
            e_new = small.tile([1, REPEATS], F16)
            attn = small.tile([128, REPEATS], F16)
            e_sb = big.tile([128, TCH, REPEATS], F16)
            t1 = small.tile([128, REPEATS], F32)
            t2 = small.tile([128, REPEATS], F32)

            with tc.tile_pool(name="ps_qkv", bufs=1, space="PSUM") as ps_qkv:
                pqT = ps_qkv.tile([128, REPEATS], F32)
                pkvT = ps_qkv.tile([128, 2], F32)
                # kv projection (Pool ring delivers wkv first); groups in one
                # psum tile must be sequential -> column-outer loops
                for t in range(2):
                    for c in range(KCH):
                        nc.tensor.matmul(
                            pkvT[:, t:t + 1],
                            wkv_sb[:, c * NKV + t * 128:c * NKV + (t + 1) * 128],
                            x_sb[:, c:c + 1],
                            start=(c == 0), stop=(c == KCH - 1),
                        )
                for qb in range(REPEATS):
                    for c in range(KCH):
                        nc.tensor.matmul(
                            pqT[:, qb:qb + 1],
                            wq_sb[:, c * NQ + qb * 128:c * NQ + (qb + 1) * 128],
                            x_sb[:, c:c + 1],
                            start=(c == 0), stop=(c == KCH - 1),
                        )

                # --- RoPE in column space ---
                # new k (true scale via cos/sin columns pre-divided by 64)
                nc.vector.tensor_copy(kcol[:], pkvT[:, 0:1])
                pswk = ps_qkv.tile([128, 1], F32)
                nc.tensor.matmul(pswk[:], c_sb[:, C_PSWP:C_PSWP + 128],
                                 kcol[:], start=True, stop=True)
                nc.vector.tensor_copy(swk[:], pswk[:])
                nc.vector.tensor_mul(t1[:, 0:1], kcol[:], c_sb[:, C_CK:C_CK + 1])
                nc.vector.tensor_mul(t2[:, 0:1], swk[:], c_sb[:, C_SK:C_SK + 1])
                nc.vector.tensor_add(knT[:], t1[:, 0:1], t2[:, 0:1])
                # new v: true scale, as a row for the rank-1 AV matmul
                nc.vector.tensor_scalar_mul(vcol[:], pkvT[:, 1:2], 1.0 / WS)
                pxv = ps_qkv.tile([1, HEAD_DIM], F16)
                nc.tensor.transpose(pxv[:], vcol[:], c_sb[:, C_ID:C_ID + 128])
                nc.vector.tensor_copy(xv_sb[:], pxv[:])
                # q heads
                nc.vector.tensor_copy(qTpre[:], pqT[:])
                pswq = ps_qkv.tile([128, REPEATS], F32)
                nc.tensor.matmul(pswq[:], c_sb[:, C_PSWP:C_PSWP + 128],
                                 qTpre[:], start=True, stop=True)
                nc.vector.tensor_copy(swq[:], pswq[:])
                nc.vector.tensor_mul(t1[:], qTpre[:], c_sb[:, C_CQ:C_CQ + REPEATS])
                nc.vector.tensor_mul(t2[:], swq[:], c_sb[:, C_SQ:C_SQ + REPEATS])
                nc.vector.tensor_add(qT[:], t1[:], t2[:])

            with tc.tile_pool(name="ps_att", bufs=1, space="PSUM") as ps_att:
                pscore = ps_att.tile([128, TCH * REPEATS], F32)
                psnew = ps_att.tile([1, REPEATS], F32)
                pav = ps_att.tile([128, REPEATS], F32)
                psv = pscore[:].rearrange("p (j h) -> p j h", h=REPEATS)
                exps = []
                for g in range(2):
                    for j in range(g * (TCH // 2), (g + 1) * (TCH // 2)):
                        nc.tensor.matmul(
                            pscore[:, j * REPEATS:(j + 1) * REPEATS],
                            kt_sb[:, j * 128:(j + 1) * 128],
                            qT[:],
                            start=True, stop=True,
                        )
                    exps.append(nc.scalar.activation(
                        e_sb[:, g * (TCH // 2):(g + 1) * (TCH // 2), :],
                        psv[:, g * (TCH // 2):(g + 1) * (TCH // 2), :],
                        mybir.ActivationFunctionType.Exp,
                        scale=float(SCALE),
                    ))
                # new-token score/exp (decoupled from the cache stream)
                nc.tensor.matmul(psnew[:], knT[:], qT[:], start=True, stop=True)
                exps.append(nc.scalar.activation(
                    e_new[:], psnew[:],
                    mybir.ActivationFunctionType.Exp, scale=float(SCALE),
                ))
                # zero the stale cache slot (rolled to partition 0)
                nc.vector.memset(e_sb[0:1, TCH - 1, :], 0.0)

                # ACT ring is now past its exps: fetch its wo stripe (the
                # nosync deps pin the scheduler to this order — exps must not
                # queue behind the wo transfer on the serial ACT engine)
                for ring, lo, hi in STRIPES["wo"]:
                    if ring == 1:
                        d = rings[ring].dma_start(
                            out=wo_sb[:, lo:hi], in_=wo8[:, lo:hi])
                        for e in exps:
                            add_dep_helper(d.ins, e.ins, sync=False,
                                           reason="exp before ACT wo stripe")
                        chain(ring, d)

                # attn_T [128 d, 4 h]: rank-1 new-token term, then cache
                nc.tensor.matmul(pav[:], xv_sb[:], e_new[:],
                                 start=True, stop=False)
                for j in range(TCH):
                    nc.tensor.matmul(
                        pav[:], v_sb[:, j * 128:(j + 1) * 128], e_sb[:, j, :],
                        start=False, stop=(j == TCH - 1),
                    )

                # normalize: pz = 64*(z_cache + e_new); attn = pav / pz
                zpart = small.tile([128, REPEATS], F32)
                ev = e_sb[:].rearrange("p j h -> p h j")
                nc.vector.reduce_sum(zpart[:], ev[:], axis=mybir.AxisListType.X)
                pz = ps_att.tile([1, REPEATS], F32)
                nc.tensor.matmul(pz[:], ones_sb[:], zpart[:],
                                 start=True, stop=False)
                nc.tensor.matmul(pz[:], ones1[:], e_new[:],
                                 start=False, stop=True)
                rz = small.tile([1, REPEATS], F32)
                nc.vector.reciprocal(rz[:], pz[:])
                przb = ps_att.tile([128, REPEATS], F32)
                nc.tensor.matmul(przb[:], ones_row[:], rz[:], start=True, stop=True)
                rzb_sb = small.tile([128, REPEATS], F32)
                nc.vector.tensor_copy(rzb_sb[:], przb[:])
                nc.vector.tensor_mul(attn[:], pav[:], rzb_sb[:])

            # --- output projection: stationary wo tiles [128 d, 128 n] ---
            o_sb = small.tile([128, NB], F32)
            with tc.tile_pool(name="ps_o", bufs=1, space="PSUM") as ps_o:
                pout = ps_o.tile([128, NB], F32)
                for m in range(NB):
                    for g in range(REPEATS):
                        nc.tensor.matmul(
                            pout[:, m:m + 1],
                            wo_sb[:, m * NQ + g * 128:m * NQ + (g + 1) * 128],
                            attn[:, g:g + 1],
                            start=(g == 0), stop=(g == REPEATS - 1),
                        )
                    if m % 8 == 7:
                        r = m // 8
                        nc.vector.tensor_copy(
                            o_sb[:, 8 * r:8 * r + 8], pout[:, 8 * r:8 * r + 8])
                        if m == 15:
                            nc.sync.dma_start(out=out_p[:, :16], in_=o_sb[:, :16])
                        elif m == 31:
                            nc.sync.dma_start(out=out_p[:, 16:], in_=o_sb[:, 16:])

    nc.compile()
    return nc


def _shard_inputs(x, wq, wk, wv, wo, cache_k, cache_v, cos, sin):
    """Build the 8 per-core input maps (fp16/fp8e3, C-contiguous)."""
    x16 = np.asarray(x, dtype=np.float32).reshape(DIM).astype(np.float16)
    x_col = x16.reshape(KCH, 128).T                   # [128, 32]
    cos_f = np.asarray(cos, np.float32).reshape(-1)   # [64]
    sin_f = np.asarray(sin, np.float32).reshape(-1)
    # column rope coefficients: out[d] = in[d]*c[d//2] + in[d^1]*s'[d],
    # s'[2i] = -s[i], s'[2i+1] = +s[i]
    ccol = np.repeat(cos_f, 2)                        # [128]
    scol = np.repeat(sin_f, 2) * np.tile([-1.0, 1.0], 64)
    pswp = np.zeros((128, 128), np.float32)
    for p in range(128):
        pswp[p, p ^ 1] = 1.0
    consts = np.zeros((128, CONST_COLS), np.float16)
    consts[:, C_PSWP:C_PSWP + 128] = pswp
    consts[:, C_CQ:C_CQ + REPEATS] = ccol[:, None]
    consts[:, C_SQ:C_SQ + REPEATS] = scol[:, None]
    consts[:, C_CK] = ccol / WS
    consts[:, C_SK] = scol / WS
    consts[:, C_X:C_X + KCH] = x_col
    consts[:, C_ID:C_ID + 128] = np.eye(128, dtype=np.float16)
    consts = np.ascontiguousarray(consts)

    wq = np.asarray(wq, np.float32)
    wk = np.asarray(wk, np.float32)
    wv = np.asarray(wv, np.float32)
    wo = np.asarray(wo, np.float32)
    cache_k = np.asarray(cache_k, np.float32)
    cache_v = np.asarray(cache_v, np.float32)

    in_maps = []
    for c in range(N_CORES):
        wq_c = wq[c * NQ:(c + 1) * NQ]                # [512, 4096]
        # stationary blocks: col c*512 + qb*128 + f = wq_c[qb*128+f, c*128+p]
        wq_m = np.ascontiguousarray(
            wq_c.reshape(REPEATS, 128, KCH, 128)      # [qb, f, cc, p]
            .transpose(3, 2, 0, 1)
            .reshape(128, KCH * NQ)
            .astype(np.float16)
        )
        kvt = np.concatenate(
            [wk[c * HEAD_DIM:(c + 1) * HEAD_DIM],
             wv[c * HEAD_DIM:(c + 1) * HEAD_DIM]]
        ) * WS                                        # [256 kv, 4096 x]
        # col c*256 + t*128 + f = kvt[t*128+f, c*128+p]
        wkv_c = np.ascontiguousarray(
            kvt.reshape(2, 128, KCH, 128)
            .transpose(3, 2, 0, 1)
            .reshape(128, KCH * NKV)
            .astype(E3M4)
        )
        W = (wo[:, c * NQ:(c + 1) * NQ] * WS)         # [4096 n, 512 gd]
        wo_c = np.ascontiguousarray(
            W.reshape(NB, 128, REPEATS, 128)          # [m, n, g, d]
            .transpose(3, 0, 2, 1)                    # [d, m, g, n]
            .reshape(128, 4 * DIM)
            .astype(E3M4)
        )
        # roll the final chunk's positions by 1 so stale position 8191 maps
        # to partition 0 (within-chunk position order is reduction-invariant)
        k_pos = cache_k[0, :KV_LEN, c, :]             # [8192, 128]
        k_pos = np.concatenate(
            [k_pos[:KV_LEN - 128], np.roll(k_pos[KV_LEN - 128:], 1, axis=0)])
        v_pos = cache_v[0, :KV_LEN, c, :]
        v_pos = np.concatenate(
            [v_pos[:KV_LEN - 128], np.roll(v_pos[KV_LEN - 128:], 1, axis=0)])
        k_c = np.ascontiguousarray(k_pos.T.astype(np.float16))
        v_c = np.ascontiguousarray(
            v_pos.reshape(TCH, 128, HEAD_DIM)
            .transpose(1, 0, 2)
            .reshape(128, TCH * 128)
            .astype(np.float16)
        )  # [128, 64*128]
        in_maps.append(
            {
                "consts": consts,
                "wq_m": wq_m,
                "wkv8": wkv_c,
                "wo8": wo_c,
                "k_t": k_c,
                "v_s": v_c,
            }
        )
    return in_maps


def get_program(reps=1):
    key = f"nc{reps}"
    if key not in _CACHED:
        _CACHED[key] = _build(reps)
    return _CACHED[key]


def kernel(x, wq, wk, wv, wo, cache_k, cache_v, cos, sin, start_pos):
    nc = get_program()
    in_maps = _shard_inputs(x, wq, wk, wv, wo, cache_k, cache_v, cos, sin)
    res = run_bass_kernel_spmd(nc, in_maps, list(range(N_CORES)))
    acc = np.zeros((128, NB), np.float32)
    for c in range(N_CORES):
        acc += res.results[c]["out_p"]
    return np.ascontiguousarray(acc.T.reshape(1, 1, DIM))


# revision 46
# speedup vs baseline: 1.0228x; 1.0228x over previous
"""Single-token GQA decode attention (32 q heads / 8 kv heads, 8192-pos KV
cache, dim 4096) tensor-parallel over 8 NeuronCores.

Sharding (per core c): q heads [4c, 4c+4), kv head c; x replicated; each
core computes a full-width [128, 32] partial of the output projection
(out[n] lives at [n % 128, n // 128]); partials are summed and transposed
host-side.

Layouts are chosen so every matmul is stationary-weight with a tiny moving
side (out free size 1-4, ~free on the PE):
  - wq  fp16 [128, 16384]: block (c, qb) at cols c*512+qb*128 is the
    [x-dim 128, q-dim 128] stationary tile; pqT accumulates q in column
    (transposed) layout [128 d, 4 h] directly.
  - wkv fp8e3 x64 [128, 8192]: same per-c blocks for the new k|v columns.
  - wo  fp8e3 x64 [128, 16384]: block (m, g) at cols m*512+g*128 maps
    attn column g to outputs [m*128, (m+1)*128).
  - K   fp16 [128, 8192] transposed cache, V fp16 [128, 64, 128]
    partition-swizzled cache (fp16: K/V quantization noise is amplified
    through the softmax exponent / feeds the output directly).
RoPE is applied in column space: q_rot = cos_col * q + sin_col' * (P q)
with P a constant pair-swap matrix (one matmul + 3 DVE ops); the k-new
rotation folds the 1/64 fp8 weight scale into its cos/sin columns.

The new token is decoupled from the cache stream: the stale cache slot at
position 8191 (rolled to partition 0 of the last chunk host-side) flows
through scores/exp and its e is zeroed; the real new-token contribution
enters via rank-1 matmuls (e_new, pav += xv^T e_new, pz += 64*e_new).

DMA: the cost model charges each engine ring independently, so the load
stream is striped across the three DMA-capable rings (SP, Activation,
Pool/gpsimd), each ~1/3 of the ~88KB of per-partition line bytes.  All
small constants ship as one [128, 170] fp16 tensor.  ACT carries less DMA
because it owns LoadActFuncSet and the exp ops, which must slot between
its stripes before AV.
"""

import numpy as np
import ml_dtypes

import concourse.tile as tile
from concourse import bacc, mybir
from concourse.bass_utils import run_bass_kernel_spmd
from concourse.tile import add_dep_helper

N_CORES = 8
DIM = 4096
HEAD_DIM = 128
N_HEADS = 32
N_KV_HEADS = 8
REPEATS = N_HEADS // N_KV_HEADS  # 4 q heads per core
KV_LEN = 8192                    # start_pos + 1
NQ = REPEATS * HEAD_DIM          # 512 local q dims
NKV = 2 * HEAD_DIM               # 256 local k|v dims
KCH = DIM // 128                 # 32 contraction chunks
TCH = KV_LEN // 128              # 64 kv-position chunks
NB = DIM // 128                  # 32 output n-blocks
WS = 64.0                        # fp8 weight scale (power of 2, exact)
SCALE = 1.0 / np.sqrt(np.float32(HEAD_DIM))

F32 = mybir.dt.float32
F16 = mybir.dt.float16
F8 = mybir.dt.float8e3
E3M4 = ml_dtypes.float8_e3m4

# const tensor column layout (fp16 [128, CONST_COLS])
C_PSWP = 0            # [128, 128] pair-swap permutation
C_CQ = 128            # [128, 4] cos column for q
C_SQ = 132            # [128, 4] signed sin column for q
C_CK = 136            # [128, 1] cos column / 64 for new k
C_SK = 137            # [128, 1] signed sin column / 64 for new k
C_X = 138             # [128, 32] x columns
C_ID = 170            # [128, 128] identity (moving side of col->row transpose)
CONST_COLS = 298

# DMA striping: per tensor, list of (ring, lo, hi) column ranges.
# ring 0 = SP, 1 = ACT, 2 = Pool.  Budgets (line KB): wq 32 f16, wkv 8 f8,
# K 16 f16, V 16 f16, wo 16 f8; ACT carries less (LAFS + exps).
STRIPES = {
    # line-KB budgets: SP ~33, ACT ~29 + LAFS + exps, Pool ~34
    # wq cols (of 16384, f16): SP 14KB, ACT 10KB, Pool 8KB (Pool starts with
    # wkv, so its wq share is smallest — wq-full gates qT/scores)
    "wq": [(0, 0, 7168), (1, 7168, 12288), (2, 12288, 16384)],
    # wkv (8192 cols f8): Pool first (the kv projection heads the PE queue
    # and unblocks psnew/e_new long before the exps need to run)
    "wkv": [(2, 0, 8192)],
    # K cols (of 8192, f16): SP 5KB, ACT 6KB, Pool 5KB
    "k": [(0, 0, 2560), (1, 2560, 5632), (2, 5632, 8192)],
    # V pos-cols (of 8192, f16): SP 6KB, ACT 5KB, Pool 5KB
    "v": [(0, 0, 3072), (1, 3072, 5632), (2, 5632, 8192)],
    # wo8: contraction heads g<2 as fp8e3 (8KB): ACT after its exps, two
    # pieces so early output blocks' sems fire sooner
    "wo8": [(1, 0, 4096), (1, 4096, 8192)],
    # wo16: contraction heads g>=2 as fp16 (16KB): SP 9KB, Pool 7KB, split
    # for finer completion granularity
    "wo16": [(0, 0, 2048), (0, 2048, 4608), (2, 4608, 6656), (2, 6656, 8192)],
}

_CACHED = {}


def _build(reps=1):
    nc = bacc.Bacc(None, target_bir_lowering=False)

    consts = nc.dram_tensor("consts", [128, CONST_COLS], F16, kind="ExternalInput")
    wq_m = nc.dram_tensor("wq_m", [128, KCH * NQ], F16, kind="ExternalInput")
    wkv8 = nc.dram_tensor("wkv8", [128, KCH * NKV], F8, kind="ExternalInput")
    wo8 = nc.dram_tensor("wo8", [128, 2 * DIM], F8, kind="ExternalInput")
    wo16 = nc.dram_tensor("wo16", [128, 2 * DIM], F16, kind="ExternalInput")
    k_t = nc.dram_tensor("k_t", [128, KV_LEN], F16, kind="ExternalInput")
    v_s = nc.dram_tensor("v_s", [128, TCH * 128], F16, kind="ExternalInput")
    out_p = nc.dram_tensor("out_p", [128, NB], F32, kind="ExternalOutput")

    with tile.TileContext(nc) as tc:
        with (
            tc.tile_pool(name="small", bufs=1) as small,
            tc.tile_pool(name="big", bufs=1) as big,
        ):
          for _rep in range(reps):
            rings = [nc.sync, nc.scalar, nc.gpsimd]
            tails = [None, None, None]

            def chain(ring, inst):
                if tails[ring] is not None:
                    add_dep_helper(inst.ins, tails[ring].ins, sync=False,
                                   reason="ring order")
                tails[ring] = inst

            def stripe_dma(key, sbuf, dram):
                for ring, lo, hi in STRIPES[key]:
                    chain(ring, rings[ring].dma_start(
                        out=sbuf[:, lo:hi], in_=dram[:, lo:hi]))

            c_sb = small.tile([128, CONST_COLS], F16)
            wq_sb = big.tile([128, KCH * NQ], F16)
            wkv_sb = big.tile([128, KCH * NKV], F8)
            kt_sb = big.tile([128, KV_LEN], F16)
            v_sb = big.tile([128, TCH * 128], F16)
            wo8_sb = big.tile([128, 2 * DIM], F8)
            wo16_sb = big.tile([128, 2 * DIM], F16)

            # ACT: dummy activation first so LoadActFuncSet lands at t~0
            id1 = small.tile([1, 1], F16)
            nc.vector.memset(id1[:], 1.0)
            scr1 = small.tile([1, 1], F16)
            nc.scalar.activation(scr1[:], id1[:],
                                 mybir.ActivationFunctionType.Copy)

            # the three parallel load rings
            chain(0, nc.sync.dma_start(out=c_sb[:], in_=consts[:]))
            stripe_dma("wkv", wkv_sb, wkv8)
            stripe_dma("wq", wq_sb, wq_m)
            stripe_dma("k", kt_sb, k_t)
            stripe_dma("v", v_sb, v_s)
            # wo8 (on ACT) is emitted after the exp ops; wo16 loads now
            stripe_dma("wo16", wo16_sb, wo16)

            x_sb = c_sb[:, C_X:C_X + KCH]

            ones_sb = small.tile([128, 1], F32)
            nc.vector.memset(ones_sb[:], WS)     # folds 1/64 into 1/pz
            ones1 = small.tile([1, 1], F16)
            nc.vector.memset(ones1[:], WS)
            ones_row = small.tile([1, 128], F32)
            nc.vector.memset(ones_row[:], 1.0)

            qTpre = small.tile([128, REPEATS], F16)
            swq = small.tile([128, REPEATS], F16)
            qT = small.tile([128, REPEATS], F16)
            kcol = small.tile([128, 1], F16)
            swk = small.tile([128, 1], F16)
            knT = small.tile([128, 1], F16)
            vcol = small.tile([128, 1], F16)
            xv_sb = small.tile([1, HEAD_DIM], F16)
            e_new = small.tile([1, REPEATS], F16)
            attn = small.tile([128, REPEATS], F16)
            e_sb = big.tile([128, TCH, REPEATS], F16)
            t1 = small.tile([128, REPEATS], F32)
            t2 = small.tile([128, REPEATS], F32)

            # kv projection first (wkv is Pool's first stripe; this heads the
            # PE queue and unblocks the new-token path early)
            with tc.tile_pool(name="ps_q", bufs=1, space="PSUM") as ps_q:
                pkvT = ps_q.tile([128, 2], F32)
                for t in range(2):
                    for c in range(KCH):
                        nc.tensor.matmul(
                            pkvT[:, t:t + 1],
                            wkv_sb[:, c * NKV + t * 128:c * NKV + (t + 1) * 128],
                            x_sb[:, c:c + 1],
                            start=(c == 0), stop=(c == KCH - 1),
                        )
                # new k rope (1/64 scale folded into its cos/sin columns)
                nc.vector.tensor_copy(kcol[:], pkvT[:, 0:1])
                pswk = ps_q.tile([128, 1], F32)
                nc.tensor.matmul(pswk[:], c_sb[:, C_PSWP:C_PSWP + 128],
                                 kcol[:], start=True, stop=True)
                nc.vector.tensor_copy(swk[:], pswk[:])
                nc.vector.tensor_mul(t1[:, 0:1], kcol[:], c_sb[:, C_CK:C_CK + 1])
                nc.vector.tensor_mul(t2[:, 0:1], swk[:], c_sb[:, C_SK:C_SK + 1])
                nc.vector.tensor_add(knT[:], t1[:, 0:1], t2[:, 0:1])
                # new v: true scale, as a row for the rank-1 AV matmul
                nc.vector.tensor_scalar_mul(vcol[:], pkvT[:, 1:2], 1.0 / WS)
                pxv = ps_q.tile([1, HEAD_DIM], F16)
                nc.tensor.transpose(pxv[:], vcol[:], c_sb[:, C_ID:C_ID + 128])
                nc.vector.tensor_copy(xv_sb[:], pxv[:])

                # q path: stationary wq blocks -> pqT, then column rope
                pqT = ps_q.tile([128, REPEATS], F32)
                for qb in range(REPEATS):
                    for c in range(KCH):
                        nc.tensor.matmul(
                            pqT[:, qb:qb + 1],
                            wq_sb[:, c * NQ + qb * 128:c * NQ + (qb + 1) * 128],
                            x_sb[:, c:c + 1],
                            start=(c == 0), stop=(c == KCH - 1),
                        )
                nc.vector.tensor_copy(qTpre[:], pqT[:])
                pswq = ps_q.tile([128, REPEATS], F32)
                nc.tensor.matmul(pswq[:], c_sb[:, C_PSWP:C_PSWP + 128],
                                 qTpre[:], start=True, stop=True)
                nc.vector.tensor_copy(swq[:], pswq[:])
                nc.vector.tensor_mul(t1[:], qTpre[:], c_sb[:, C_CQ:C_CQ + REPEATS])
                nc.vector.tensor_mul(t2[:], swq[:], c_sb[:, C_SQ:C_SQ + REPEATS])
                nc.vector.tensor_add(qT[:], t1[:], t2[:])

            with tc.tile_pool(name="ps_att", bufs=1, space="PSUM") as ps_att:
                pscore = ps_att.tile([128, TCH * REPEATS], F32)
                psnew = ps_att.tile([1, REPEATS], F32)
                pav = ps_att.tile([128, REPEATS], F32)
                psv = pscore[:].rearrange("p (j h) -> p j h", h=REPEATS)
                # new-token score first (tiny; its exp slots between the
                # two big exps on ACT)
                nc.tensor.matmul(psnew[:], knT[:], qT[:], start=True, stop=True)
                exps = []
                for g in range(2):
                    for j in range(g * (TCH // 2), (g + 1) * (TCH // 2)):
                        nc.tensor.matmul(
                            pscore[:, j * REPEATS:(j + 1) * REPEATS],
                            kt_sb[:, j * 128:(j + 1) * 128],
                            qT[:],
                            start=True, stop=True,
                        )
                    exps.append(nc.scalar.activation(
                        e_sb[:, g * (TCH // 2):(g + 1) * (TCH // 2), :],
                        psv[:, g * (TCH // 2):(g + 1) * (TCH // 2), :],
                        mybir.ActivationFunctionType.Exp,
                        scale=float(SCALE),
                    ))
                    if g == 0:
                        exps.append(nc.scalar.activation(
                            e_new[:], psnew[:],
                            mybir.ActivationFunctionType.Exp,
                            scale=float(SCALE),
                        ))
                # zero the stale cache slot (rolled to partition 0)
                nc.vector.memset(e_sb[0:1, TCH - 1, :], 0.0)

                # ACT ring is now past its exps: fetch wo8 (the nosync deps
                # pin the scheduler to this order — exps must not queue
                # behind the wo8 transfer on the serial ACT engine)
                for ring, lo, hi in STRIPES["wo8"]:
                    d = rings[ring].dma_start(
                        out=wo8_sb[:, lo:hi], in_=wo8[:, lo:hi])
                    for e in exps:
                        add_dep_helper(d.ins, e.ins, sync=False,
                                       reason="exp before ACT wo stripe")
                    chain(ring, d)

                # attn_T [128 d, 4 h]: rank-1 new-token term, then cache
                nc.tensor.matmul(pav[:], xv_sb[:], e_new[:],
                                 start=True, stop=False)
                for j in range(TCH):
                    nc.tensor.matmul(
                        pav[:], v_sb[:, j * 128:(j + 1) * 128], e_sb[:, j, :],
                        start=False, stop=(j == TCH - 1),
                    )

                # normalize: pz = 64*(z_cache + e_new); attn = pav / pz
                zpart = small.tile([128, REPEATS], F32)
                ev = e_sb[:].rearrange("p j h -> p h j")
                nc.vector.reduce_sum(zpart[:], ev[:], axis=mybir.AxisListType.X)
                pz = ps_att.tile([1, REPEATS], F32)
                nc.tensor.matmul(pz[:], ones_sb[:], zpart[:],
                                 start=True, stop=False)
                nc.tensor.matmul(pz[:], ones1[:], e_new[:],
                                 start=False, stop=True)
                rz = small.tile([1, REPEATS], F32)
                nc.vector.reciprocal(rz[:], pz[:])
                przb = ps_att.tile([128, REPEATS], F32)
                nc.tensor.matmul(przb[:], ones_row[:], rz[:], start=True, stop=True)
                rzb_sb = small.tile([128, REPEATS], F32)
                nc.vector.tensor_copy(rzb_sb[:], przb[:])
                nc.vector.tensor_mul(attn[:], pav[:], rzb_sb[:])

            # --- output projection: stationary wo tiles [128 d, 128 n] ---
            o_sb = small.tile([128, NB], F32)
            with tc.tile_pool(name="ps_o", bufs=1, space="PSUM") as ps_o:
                pout = ps_o.tile([128, NB], F32)
                for m in range(NB):
                    for g in range(REPEATS):
                        w_sb = wo8_sb if g < 2 else wo16_sb
                        col = m * 256 + (g % 2) * 128
                        nc.tensor.matmul(
                            pout[:, m:m + 1],
                            w_sb[:, col:col + 128],
                            attn[:, g:g + 1],
                            start=(g == 0), stop=(g == REPEATS - 1),
                        )
                    if m % 8 == 7:
                        r = m // 8
                        nc.vector.tensor_copy(
                            o_sb[:, 8 * r:8 * r + 8], pout[:, 8 * r:8 * r + 8])
                        if m == 15:
                            nc.sync.dma_start(out=out_p[:, :16], in_=o_sb[:, :16])
                        elif m == 31:
                            nc.sync.dma_start(out=out_p[:, 16:], in_=o_sb[:, 16:])

    nc.compile()
    return nc


def _shard_inputs(x, wq, wk, wv, wo, cache_k, cache_v, cos, sin):
    """Build the 8 per-core input maps (fp16/fp8e3, C-contiguous)."""
    x16 = np.asarray(x, dtype=np.float32).reshape(DIM).astype(np.float16)
    x_col = x16.reshape(KCH, 128).T                   # [128, 32]
    cos_f = np.asarray(cos, np.float32).reshape(-1)   # [64]
    sin_f = np.asarray(sin, np.float32).reshape(-1)
    # column rope coefficients: out[d] = in[d]*c[d//2] + in[d^1]*s'[d],
    # s'[2i] = -s[i], s'[2i+1] = +s[i]
    ccol = np.repeat(cos_f, 2)                        # [128]
    scol = np.repeat(sin_f, 2) * np.tile([-1.0, 1.0], 64)
    pswp = np.zeros((128, 128), np.float32)
    for p in range(128):
        pswp[p, p ^ 1] = 1.0
    consts = np.zeros((128, CONST_COLS), np.float16)
    consts[:, C_PSWP:C_PSWP + 128] = pswp
    consts[:, C_CQ:C_CQ + REPEATS] = ccol[:, None]
    consts[:, C_SQ:C_SQ + REPEATS] = scol[:, None]
    consts[:, C_CK] = ccol / WS
    consts[:, C_SK] = scol / WS
    consts[:, C_X:C_X + KCH] = x_col
    consts[:, C_ID:C_ID + 128] = np.eye(128, dtype=np.float16)
    consts = np.ascontiguousarray(consts)

    wq = np.asarray(wq, np.float32)
    wk = np.asarray(wk, np.float32)
    wv = np.asarray(wv, np.float32)
    wo = np.asarray(wo, np.float32)
    cache_k = np.asarray(cache_k, np.float32)
    cache_v = np.asarray(cache_v, np.float32)

    in_maps = []
    for c in range(N_CORES):
        wq_c = wq[c * NQ:(c + 1) * NQ]                # [512, 4096]
        # stationary blocks: col c*512 + qb*128 + f = wq_c[qb*128+f, c*128+p]
        wq_m = np.ascontiguousarray(
            wq_c.reshape(REPEATS, 128, KCH, 128)      # [qb, f, cc, p]
            .transpose(3, 2, 0, 1)
            .reshape(128, KCH * NQ)
            .astype(np.float16)
        )
        kvt = np.concatenate(
            [wk[c * HEAD_DIM:(c + 1) * HEAD_DIM],
             wv[c * HEAD_DIM:(c + 1) * HEAD_DIM]]
        ) * WS                                        # [256 kv, 4096 x]
        # col c*256 + t*128 + f = kvt[t*128+f, c*128+p]
        wkv_c = np.ascontiguousarray(
            kvt.reshape(2, 128, KCH, 128)
            .transpose(3, 2, 0, 1)
            .reshape(128, KCH * NKV)
            .astype(E3M4)
        )
        W = wo[:, c * NQ:(c + 1) * NQ]                # [4096 n, 512 gd]
        Wdm = W.reshape(NB, 128, REPEATS, 128).transpose(3, 0, 2, 1)  # [d,m,g,n]
        # contraction split: heads g<2 fp8e3, g>=2 fp16 (both x64 to match
        # the 1/64 folded into attn); col = m*256 + (g%2)*128 + n
        wo8_c = np.ascontiguousarray(
            (Wdm[:, :, :2] * WS).reshape(128, 2 * DIM).astype(E3M4))
        wo16_c = np.ascontiguousarray(
            (Wdm[:, :, 2:] * WS).reshape(128, 2 * DIM).astype(np.float16))
        # roll the final chunk's positions by 1 so stale position 8191 maps
        # to partition 0 (within-chunk position order is reduction-invariant)
        k_pos = cache_k[0, :KV_LEN, c, :]             # [8192, 128]
        k_pos = np.concatenate(
            [k_pos[:KV_LEN - 128], np.roll(k_pos[KV_LEN - 128:], 1, axis=0)])
        v_pos = cache_v[0, :KV_LEN, c, :]
        v_pos = np.concatenate(
            [v_pos[:KV_LEN - 128], np.roll(v_pos[KV_LEN - 128:], 1, axis=0)])
        k_c = np.ascontiguousarray(k_pos.T.astype(np.float16))
        v_c = np.ascontiguousarray(
            v_pos.reshape(TCH, 128, HEAD_DIM)
            .transpose(1, 0, 2)
            .reshape(128, TCH * 128)
            .astype(np.float16)
        )  # [128, 64*128]
        in_maps.append(
            {
                "consts": consts,
                "wq_m": wq_m,
                "wkv8": wkv_c,
                "wo8": wo8_c,
                "wo16": wo16_c,
                "k_t": k_c,
                "v_s": v_c,
            }
        )
    return in_maps


def get_program(reps=1):
    key = f"nc{reps}"
    if key not in _CACHED:
        _CACHED[key] = _build(reps)
    return _CACHED[key]


def kernel(x, wq, wk, wv, wo, cache_k, cache_v, cos, sin, start_pos):
    nc = get_program()
    in_maps = _shard_inputs(x, wq, wk, wv, wo, cache_k, cache_v, cos, sin)
    res = run_bass_kernel_spmd(nc, in_maps, list(range(N_CORES)))
    acc = np.zeros((128, NB), np.float32)
    for c in range(N_CORES):
        acc += res.results[c]["out_p"]
    return np.ascontiguousarray(acc.T.reshape(1, 1, DIM))


# revision 49
# speedup vs baseline: 1.0511x; 1.0277x over previous
"""Single-token GQA decode attention (32 q heads / 8 kv heads, 8192-pos KV
cache, dim 4096) tensor-parallel over 8 NeuronCores.

Sharding (per core c): q heads [4c, 4c+4), kv head c; x replicated; each
core computes a full-width [128, 32] partial of the output projection
(out[n] lives at [n % 128, n // 128]); partials are summed and transposed
host-side.

Layouts are chosen so every matmul is stationary-weight with a tiny moving
side (out free size 1-4, ~free on the PE):
  - wq  fp16 [128, 16384]: block (c, qb) at cols c*512+qb*128 is the
    [x-dim 128, q-dim 128] stationary tile; pqT accumulates q in column
    (transposed) layout [128 d, 4 h] directly.
  - wkv fp8e3 x64 [128, 8192]: same per-c blocks for the new k|v columns.
  - wo  fp8e3 x64 [128, 16384]: block (m, g) at cols m*512+g*128 maps
    attn column g to outputs [m*128, (m+1)*128).
  - K   fp16 [128, 8192] transposed cache, V fp16 [128, 64, 128]
    partition-swizzled cache (fp16: K/V quantization noise is amplified
    through the softmax exponent / feeds the output directly).
RoPE is applied in column space: q_rot = cos_col * q + sin_col' * (P q)
with P a constant pair-swap matrix (one matmul + 3 DVE ops); the k-new
rotation folds the 1/64 fp8 weight scale into its cos/sin columns.

The new token is decoupled from the cache stream: the stale cache slot at
position 8191 (rolled to partition 0 of the last chunk host-side) flows
through scores/exp and its e is zeroed; the real new-token contribution
enters via rank-1 matmuls (e_new, pav += xv^T e_new, pz += 64*e_new).

DMA: the cost model charges each engine ring independently, so the load
stream is striped across the three DMA-capable rings (SP, Activation,
Pool/gpsimd), each ~1/3 of the ~88KB of per-partition line bytes.  All
small constants ship as one [128, 170] fp16 tensor.  ACT carries less DMA
because it owns LoadActFuncSet and the exp ops, which must slot between
its stripes before AV.
"""

import numpy as np
import ml_dtypes

import concourse.tile as tile
from concourse import bacc, mybir
from concourse.bass_utils import run_bass_kernel_spmd
from concourse.tile import add_dep_helper

N_CORES = 8
DIM = 4096
HEAD_DIM = 128
N_HEADS = 32
N_KV_HEADS = 8
REPEATS = N_HEADS // N_KV_HEADS  # 4 q heads per core
KV_LEN = 8192                    # start_pos + 1
NQ = REPEATS * HEAD_DIM          # 512 local q dims
NKV = 2 * HEAD_DIM               # 256 local k|v dims
KCH = DIM // 128                 # 32 contraction chunks
TCH = KV_LEN // 128              # 64 kv-position chunks
NB = DIM // 128                  # 32 output n-blocks
WS = 64.0                        # fp8 weight scale (power of 2, exact)
SCALE = 1.0 / np.sqrt(np.float32(HEAD_DIM))

F32 = mybir.dt.float32
F16 = mybir.dt.float16
F8 = mybir.dt.float8e3
E3M4 = ml_dtypes.float8_e3m4

# const tensor column layout (fp16 [128, CONST_COLS])
C_PSWP = 0            # [128, 128] pair-swap permutation
C_CQ = 128            # [128, 4] cos column for q
C_SQ = 132            # [128, 4] signed sin column for q
C_CK = 136            # [128, 1] cos column / 64 for new k
C_SK = 137            # [128, 1] signed sin column / 64 for new k
C_X = 138             # [128, 32] x columns
C_ID = 170            # [128, 128] identity (moving side of col->row transpose)
CONST_COLS = 298

# DMA striping: per tensor, list of (ring, lo, hi) column ranges.
# ring 0 = SP, 1 = ACT, 2 = Pool.  Budgets (line KB): wq 32 f16, wkv 8 f8,
# K 16 f16, V 16 f16, wo 16 f8; ACT carries less (LAFS + exps).
STRIPES = {
    # line-KB budgets: SP ~33, ACT ~29 + LAFS + exps, Pool ~34
    # wq cols (of 16384, f16): SP 14KB, ACT 10KB, Pool 8KB (Pool starts with
    # wkv, so its wq share is smallest — wq-full gates qT/scores)
    "wq": [(0, 0, 7168), (1, 7168, 12288), (2, 12288, 16384)],
    # wkv (8192 cols f8): Pool first (the kv projection heads the PE queue
    # and unblocks psnew/e_new long before the exps need to run)
    "wkv": [(2, 0, 8192)],
    # K cols (of 8192, f16): SP 5KB, ACT 6KB, Pool 5KB
    "k": [(0, 0, 2560), (1, 2560, 5632), (2, 5632, 8192)],
    # V pos-cols (of 8192, f16): SP 6KB, ACT 5KB, Pool 5KB
    "v": [(0, 0, 3072), (1, 3072, 5632), (2, 5632, 8192)],
    # wo8: contraction heads g<2 as fp8e3 (8KB): ACT after its exps, two
    # pieces so early output blocks' sems fire sooner
    "wo8": [(1, 0, 4096), (1, 4096, 8192)],
    # wo16: contraction heads g>=2 as fp16 (16KB): SP 9KB, Pool 7KB, split
    # for finer completion granularity
    "wo16": [(0, 0, 2048), (0, 2048, 4864), (2, 4864, 6656), (2, 6656, 8192)],
}

_CACHED = {}


def _build(reps=1):
    nc = bacc.Bacc(None, target_bir_lowering=False)

    consts = nc.dram_tensor("consts", [128, CONST_COLS], F16, kind="ExternalInput")
    wq_m = nc.dram_tensor("wq_m", [128, KCH * NQ], F16, kind="ExternalInput")
    wkv8 = nc.dram_tensor("wkv8", [128, KCH * NKV], F8, kind="ExternalInput")
    wo8 = nc.dram_tensor("wo8", [128, 2 * DIM], F8, kind="ExternalInput")
    wo16 = nc.dram_tensor("wo16", [128, 2 * DIM], F16, kind="ExternalInput")
    k_t = nc.dram_tensor("k_t", [128, KV_LEN], F16, kind="ExternalInput")
    v_s = nc.dram_tensor("v_s", [128, TCH * 128], F16, kind="ExternalInput")
    out_p = nc.dram_tensor("out_p", [128, NB], F32, kind="ExternalOutput")

    with tile.TileContext(nc) as tc:
        with (
            tc.tile_pool(name="small", bufs=1) as small,
            tc.tile_pool(name="big", bufs=1) as big,
        ):
          for _rep in range(reps):
            rings = [nc.sync, nc.scalar, nc.gpsimd]
            tails = [None, None, None]

            def chain(ring, inst):
                if tails[ring] is not None:
                    add_dep_helper(inst.ins, tails[ring].ins, sync=False,
                                   reason="ring order")
                tails[ring] = inst

            def stripe_dma(key, sbuf, dram):
                for ring, lo, hi in STRIPES[key]:
                    chain(ring, rings[ring].dma_start(
                        out=sbuf[:, lo:hi], in_=dram[:, lo:hi]))

            c_sb = small.tile([128, CONST_COLS], F16)
            wq_sb = big.tile([128, KCH * NQ], F16)
            wkv_sb = big.tile([128, KCH * NKV], F8)
            kt_sb = big.tile([128, KV_LEN], F16)
            v_sb = big.tile([128, TCH * 128], F16)
            wo8_sb = big.tile([128, 2 * DIM], F8)
            wo16_sb = big.tile([128, 2 * DIM], F16)

            # ACT: dummy activation first so LoadActFuncSet lands at t~0
            id1 = small.tile([1, 1], F16)
            nc.vector.memset(id1[:], 1.0)
            scr1 = small.tile([1, 1], F16)
            nc.scalar.activation(scr1[:], id1[:],
                                 mybir.ActivationFunctionType.Copy)

            # the three parallel load rings; consts ride Pool (starts at
            # t=100) so SP's wq stripe — which gates qT — begins immediately
            chain(2, nc.gpsimd.dma_start(out=c_sb[:], in_=consts[:]))
            stripe_dma("wkv", wkv_sb, wkv8)
            stripe_dma("wq", wq_sb, wq_m)
            stripe_dma("k", kt_sb, k_t)
            stripe_dma("v", v_sb, v_s)
            # wo8 (on ACT) is emitted after the exp ops; wo16 loads now
            stripe_dma("wo16", wo16_sb, wo16)

            x_sb = c_sb[:, C_X:C_X + KCH]

            ones_sb = small.tile([128, 1], F32)
            nc.vector.memset(ones_sb[:], WS)     # folds 1/64 into 1/pz
            ones1 = small.tile([1, 1], F16)
            nc.vector.memset(ones1[:], WS)
            ones_row = small.tile([1, 128], F32)
            nc.vector.memset(ones_row[:], 1.0)

            qTpre = small.tile([128, REPEATS], F16)
            swq = small.tile([128, REPEATS], F16)
            qT = small.tile([128, REPEATS], F16)
            kcol = small.tile([128, 1], F16)
            swk = small.tile([128, 1], F16)
            knT = small.tile([128, 1], F16)
            vcol = small.tile([128, 1], F16)
            xv_sb = small.tile([1, HEAD_DIM], F16)
            e_new = small.tile([1, REPEATS], F16)
            attn = small.tile([128, REPEATS], F16)
            e_sb = big.tile([128, TCH, REPEATS], F16)
            t1 = small.tile([128, REPEATS], F32)
            t2 = small.tile([128, REPEATS], F32)

            # kv projection first (wkv is Pool's first stripe; this heads the
            # PE queue and unblocks the new-token path early)
            with tc.tile_pool(name="ps_q", bufs=1, space="PSUM") as ps_q:
                pkvT = ps_q.tile([128, 2], F32)
                for t in range(2):
                    for c in range(KCH):
                        nc.tensor.matmul(
                            pkvT[:, t:t + 1],
                            wkv_sb[:, c * NKV + t * 128:c * NKV + (t + 1) * 128],
                            x_sb[:, c:c + 1],
                            start=(c == 0), stop=(c == KCH - 1),
                        )
                # new k rope (1/64 scale folded into its cos/sin columns)
                nc.vector.tensor_copy(kcol[:], pkvT[:, 0:1])
                pswk = ps_q.tile([128, 1], F32)
                nc.tensor.matmul(pswk[:], c_sb[:, C_PSWP:C_PSWP + 128],
                                 kcol[:], start=True, stop=True)
                nc.vector.tensor_copy(swk[:], pswk[:])
                nc.vector.tensor_mul(t1[:, 0:1], kcol[:], c_sb[:, C_CK:C_CK + 1])
                nc.vector.tensor_mul(t2[:, 0:1], swk[:], c_sb[:, C_SK:C_SK + 1])
                nc.vector.tensor_add(knT[:], t1[:, 0:1], t2[:, 0:1])
                # new v: true scale, as a row for the rank-1 AV matmul
                nc.vector.tensor_scalar_mul(vcol[:], pkvT[:, 1:2], 1.0 / WS)
                pxv = ps_q.tile([1, HEAD_DIM], F16)
                nc.tensor.transpose(pxv[:], vcol[:], c_sb[:, C_ID:C_ID + 128])
                nc.vector.tensor_copy(xv_sb[:], pxv[:])

                # q path: stationary wq blocks -> pqT, then column rope
                pqT = ps_q.tile([128, REPEATS], F32)
                for qb in range(REPEATS):
                    for c in range(KCH):
                        nc.tensor.matmul(
                            pqT[:, qb:qb + 1],
                            wq_sb[:, c * NQ + qb * 128:c * NQ + (qb + 1) * 128],
                            x_sb[:, c:c + 1],
                            start=(c == 0), stop=(c == KCH - 1),
                        )
                nc.vector.tensor_copy(qTpre[:], pqT[:])
                pswq = ps_q.tile([128, REPEATS], F32)
                nc.tensor.matmul(pswq[:], c_sb[:, C_PSWP:C_PSWP + 128],
                                 qTpre[:], start=True, stop=True)
                nc.vector.tensor_copy(swq[:], pswq[:])
                nc.vector.tensor_mul(t1[:], qTpre[:], c_sb[:, C_CQ:C_CQ + REPEATS])
                nc.vector.tensor_mul(t2[:], swq[:], c_sb[:, C_SQ:C_SQ + REPEATS])
                nc.vector.tensor_add(qT[:], t1[:], t2[:])

            with tc.tile_pool(name="ps_att", bufs=1, space="PSUM") as ps_att:
                pscore = ps_att.tile([128, TCH * REPEATS], F32)
                psnew = ps_att.tile([1, REPEATS], F32)
                pav = ps_att.tile([128, REPEATS], F32)
                psv = pscore[:].rearrange("p (j h) -> p j h", h=REPEATS)
                # new-token score first (tiny; its exp slots between the
                # two big exps on ACT)
                nc.tensor.matmul(psnew[:], knT[:], qT[:], start=True, stop=True)
                exps = []
                for g in range(2):
                    for j in range(g * (TCH // 2), (g + 1) * (TCH // 2)):
                        nc.tensor.matmul(
                            pscore[:, j * REPEATS:(j + 1) * REPEATS],
                            kt_sb[:, j * 128:(j + 1) * 128],
                            qT[:],
                            start=True, stop=True,
                        )
                    exps.append(nc.scalar.activation(
                        e_sb[:, g * (TCH // 2):(g + 1) * (TCH // 2), :],
                        psv[:, g * (TCH // 2):(g + 1) * (TCH // 2), :],
                        mybir.ActivationFunctionType.Exp,
                        scale=float(SCALE),
                    ))
                    if g == 0:
                        exps.append(nc.scalar.activation(
                            e_new[:], psnew[:],
                            mybir.ActivationFunctionType.Exp,
                            scale=float(SCALE),
                        ))
                # zero the stale cache slot (rolled to partition 0)
                nc.vector.memset(e_sb[0:1, TCH - 1, :], 0.0)

                # ACT ring is now past its exps: fetch wo8 (the nosync deps
                # pin the scheduler to this order — exps must not queue
                # behind the wo8 transfer on the serial ACT engine)
                for ring, lo, hi in STRIPES["wo8"]:
                    d = rings[ring].dma_start(
                        out=wo8_sb[:, lo:hi], in_=wo8[:, lo:hi])
                    for e in exps:
                        add_dep_helper(d.ins, e.ins, sync=False,
                                       reason="exp before ACT wo stripe")
                    chain(ring, d)

                # attn_T [128 d, 4 h]: rank-1 new-token term, then cache
                nc.tensor.matmul(pav[:], xv_sb[:], e_new[:],
                                 start=True, stop=False)
                for j in range(TCH):
                    nc.tensor.matmul(
                        pav[:], v_sb[:, j * 128:(j + 1) * 128], e_sb[:, j, :],
                        start=False, stop=(j == TCH - 1),
                    )

                # normalize: pz = 64*(z_cache + e_new); attn = pav / pz
                zpart = small.tile([128, REPEATS], F32)
                ev = e_sb[:].rearrange("p j h -> p h j")
                nc.vector.reduce_sum(zpart[:], ev[:], axis=mybir.AxisListType.X)
                pz = ps_att.tile([1, REPEATS], F32)
                nc.tensor.matmul(pz[:], ones_sb[:], zpart[:],
                                 start=True, stop=False)
                nc.tensor.matmul(pz[:], ones1[:], e_new[:],
                                 start=False, stop=True)
                rz = small.tile([1, REPEATS], F32)
                nc.vector.reciprocal(rz[:], pz[:])
                przb = ps_att.tile([128, REPEATS], F32)
                nc.tensor.matmul(przb[:], ones_row[:], rz[:], start=True, stop=True)
                rzb_sb = small.tile([128, REPEATS], F32)
                nc.vector.tensor_copy(rzb_sb[:], przb[:])
                nc.vector.tensor_mul(attn[:], pav[:], rzb_sb[:])

            # --- output projection: stationary wo tiles [128 d, 128 n] ---
            o_sb = small.tile([128, NB], F32)
            with tc.tile_pool(name="ps_o", bufs=1, space="PSUM") as ps_o:
                pout = ps_o.tile([128, NB], F32)
                for m in range(NB):
                    for g in range(REPEATS):
                        w_sb = wo8_sb if g < 2 else wo16_sb
                        col = m * 256 + (g % 2) * 128
                        nc.tensor.matmul(
                            pout[:, m:m + 1],
                            w_sb[:, col:col + 128],
                            attn[:, g:g + 1],
                            start=(g == 0), stop=(g == REPEATS - 1),
                        )
                    if m == 15:
                        nc.vector.tensor_copy(o_sb[:, :16], pout[:, :16])
                        nc.sync.dma_start(out=out_p[:, :16], in_=o_sb[:, :16])
                    elif m == 31:
                        nc.vector.tensor_copy(o_sb[:, 16:], pout[:, 16:])
                        nc.sync.dma_start(out=out_p[:, 16:], in_=o_sb[:, 16:])

    nc.compile()
    return nc


def _shard_inputs(x, wq, wk, wv, wo, cache_k, cache_v, cos, sin):
    """Build the 8 per-core input maps (fp16/fp8e3, C-contiguous)."""
    x16 = np.asarray(x, dtype=np.float32).reshape(DIM).astype(np.float16)
    x_col = x16.reshape(KCH, 128).T                   # [128, 32]
    cos_f = np.asarray(cos, np.float32).reshape(-1)   # [64]
    sin_f = np.asarray(sin, np.float32).reshape(-1)
    # column rope coefficients: out[d] = in[d]*c[d//2] + in[d^1]*s'[d],
    # s'[2i] = -s[i], s'[2i+1] = +s[i]
    ccol = np.repeat(cos_f, 2)                        # [128]
    scol = np.repeat(sin_f, 2) * np.tile([-1.0, 1.0], 64)
    pswp = np.zeros((128, 128), np.float32)
    for p in range(128):
        pswp[p, p ^ 1] = 1.0
    consts = np.zeros((128, CONST_COLS), np.float16)
    consts[:, C_PSWP:C_PSWP + 128] = pswp
    consts[:, C_CQ:C_CQ + REPEATS] = ccol[:, None]
    consts[:, C_SQ:C_SQ + REPEATS] = scol[:, None]
    consts[:, C_CK] = ccol / WS
    consts[:, C_SK] = scol / WS
    consts[:, C_X:C_X + KCH] = x_col
    consts[:, C_ID:C_ID + 128] = np.eye(128, dtype=np.float16)
    consts = np.ascontiguousarray(consts)

    wq = np.asarray(wq, np.float32)
    wk = np.asarray(wk, np.float32)
    wv = np.asarray(wv, np.float32)
    wo = np.asarray(wo, np.float32)
    cache_k = np.asarray(cache_k, np.float32)
    cache_v = np.asarray(cache_v, np.float32)

    in_maps = []
    for c in range(N_CORES):
        wq_c = wq[c * NQ:(c + 1) * NQ]                # [512, 4096]
        # stationary blocks: col c*512 + qb*128 + f = wq_c[qb*128+f, c*128+p]
        wq_m = np.ascontiguousarray(
            wq_c.reshape(REPEATS, 128, KCH, 128)      # [qb, f, cc, p]
            .transpose(3, 2, 0, 1)
            .reshape(128, KCH * NQ)
            .astype(np.float16)
        )
        kvt = np.concatenate(
            [wk[c * HEAD_DIM:(c + 1) * HEAD_DIM],
             wv[c * HEAD_DIM:(c + 1) * HEAD_DIM]]
        ) * WS                                        # [256 kv, 4096 x]
        # col c*256 + t*128 + f = kvt[t*128+f, c*128+p]
        wkv_c = np.ascontiguousarray(
            kvt.reshape(2, 128, KCH, 128)
            .transpose(3, 2, 0, 1)
            .reshape(128, KCH * NKV)
            .astype(E3M4)
        )
        W = wo[:, c * NQ:(c + 1) * NQ]                # [4096 n, 512 gd]
        Wdm = W.reshape(NB, 128, REPEATS, 128).transpose(3, 0, 2, 1)  # [d,m,g,n]
        # contraction split: heads g<2 fp8e3, g>=2 fp16 (both x64 to match
        # the 1/64 folded into attn); col = m*256 + (g%2)*128 + n
        wo8_c = np.ascontiguousarray(
            (Wdm[:, :, :2] * WS).reshape(128, 2 * DIM).astype(E3M4))
        wo16_c = np.ascontiguousarray(
            (Wdm[:, :, 2:] * WS).reshape(128, 2 * DIM).astype(np.float16))
        # roll the final chunk's positions by 1 so stale position 8191 maps
        # to partition 0 (within-chunk position order is reduction-invariant)
        k_pos = cache_k[0, :KV_LEN, c, :]             # [8192, 128]
        k_pos = np.concatenate(
            [k_pos[:KV_LEN - 128], np.roll(k_pos[KV_LEN - 128:], 1, axis=0)])
        v_pos = cache_v[0, :KV_LEN, c, :]
        v_pos = np.concatenate(
            [v_pos[:KV_LEN - 128], np.roll(v_pos[KV_LEN - 128:], 1, axis=0)])
        k_c = np.ascontiguousarray(k_pos.T.astype(np.float16))
        v_c = np.ascontiguousarray(
            v_pos.reshape(TCH, 128, HEAD_DIM)
            .transpose(1, 0, 2)
            .reshape(128, TCH * 128)
            .astype(np.float16)
        )  # [128, 64*128]
        in_maps.append(
            {
                "consts": consts,
                "wq_m": wq_m,
                "wkv8": wkv_c,
                "wo8": wo8_c,
                "wo16": wo16_c,
                "k_t": k_c,
                "v_s": v_c,
            }
        )
    return in_maps


def get_program(reps=1):
    key = f"nc{reps}"
    if key not in _CACHED:
        _CACHED[key] = _build(reps)
    return _CACHED[key]


def kernel(x, wq, wk, wv, wo, cache_k, cache_v, cos, sin, start_pos):
    nc = get_program()
    in_maps = _shard_inputs(x, wq, wk, wv, wo, cache_k, cache_v, cos, sin)
    res = run_bass_kernel_spmd(nc, in_maps, list(range(N_CORES)))
    acc = np.zeros((128, NB), np.float32)
    for c in range(N_CORES):
        acc += res.results[c]["out_p"]
    return np.ascontiguousarray(acc.T.reshape(1, 1, DIM))
